# revision 17
# baseline (speedup 1.0000x reference)
"""Multi-head attention (B=2, S=2048, H=1024, 16 heads, RoPE) on 8 trn2 cores.

Sharding: core = (batch b, head-group g); b = core // 4, g = core % 4.
Each core computes 4 heads' attention for one batch and a partial output
projection; the host sums the 4 partials per batch.

All operand data is bf16 (PE runs 1 cycle/row at any K, so no zero-padded
K=128 trick is needed for the 64-dim per-head score contractions); PSUM
accumulation stays fp32 and the output partials are written fp32.
Attention scores are computed directly in transposed [k, q] layout so the
attn @ V contraction needs no transposes; softmax normalization is deferred:
V carries an extra ones-column so the attention matmul also produces the
softmax denominator, and gpsimd partition_broadcast replicates 1/rowsum
across partitions for the final scale.

Schedule: phase B emits q(m=0), k(m=0), v, q(m=1), k(m=1) so phase C's first
score items (heads 0/1) have their rope chains complete by the time the PE
reaches them; xT is DMA'd in per-512-column chunks interleaved with the
weight loads so the first projection matmuls start ~4us in. xT and all
weights live in persistent SBUF tiles so the next rep's loads overlap this
rep's attention phase instead of waiting for pool reuse.
"""
import sys

import numpy as np
import ml_dtypes

sys.path.insert(0, "/opt/trn_rl_repo")

import concourse.bass as bass  # noqa: E402
import concourse.mybir as mybir  # noqa: E402
import concourse.tile as tile  # noqa: E402
from concourse import bacc  # noqa: E402
from concourse.bass_utils import run_bass_kernel_spmd  # noqa: E402

F32 = mybir.dt.float32
BF16 = mybir.dt.bfloat16
EXP = mybir.ActivationFunctionType.Exp

B, S, H = 2, 2048, 1024
NH, D = 16, 64                  # heads, head dim
GH = 4                          # heads per core (group)
GD = GH * D                     # 256 out dims per core
KT = H // 128                   # 8 contraction tiles for projections
MC = S // 128                   # 16 seq chunks of 128
QB = S // 512                   # 4 query blocks of 512
ROPE_BASE = 10000.0
SCALE = D ** -0.5


def _rope_tables():
    inv_freq = 1.0 / (ROPE_BASE ** (np.arange(0, D, 2, dtype=np.float64) / D))
    t = np.arange(S, dtype=np.float64)
    freqs = np.outer(t, inv_freq)                     # (S, 32)
    emb = np.concatenate([freqs, freqs], axis=-1)     # (S, 64)
    cosT = np.cos(emb).T.astype(np.float32)           # (64, S)
    sinT = np.sin(emb).T.astype(np.float32)           # (64, S)
    # sinrs is laid out at SOURCE row positions so that tmp[dest] =
    # st[src] * sinrs[src] has equal input base partitions (ISA rule):
    #   dest 0-31  <- src 32-63: factor -sin[dest]; stored at rows 32-63
    #   dest 32-63 <- src 0-31:  factor +sin[dest]; stored at rows 0-31
    # (sinT rows 0-31 and 32-63 are identical, so signs are what matter)
    sinrs = np.empty_like(sinT)
    sinrs[0:32] = sinT[0:32]
    sinrs[32:64] = -sinT[32:64]
    cos2 = np.tile(cosT, (2, 1))                      # (128, S) two heads/chunk
    sinr2 = np.tile(sinrs, (2, 1))
    bf = ml_dtypes.bfloat16
    return np.ascontiguousarray(cos2).astype(bf), np.ascontiguousarray(sinr2).astype(bf)


def _build_nc():
    nc = bacc.Bacc("TRN2", target_bir_lowering=False)
    xT = nc.dram_tensor("xT", [128, KT, S], BF16, kind="ExternalInput")
    wqT = nc.dram_tensor("wqT", [128, KT, GD], BF16, kind="ExternalInput")
    wkT = nc.dram_tensor("wkT", [128, KT, GD], BF16, kind="ExternalInput")
    wvT = nc.dram_tensor("wvT", [128, KT, GD], BF16, kind="ExternalInput")
    woT = nc.dram_tensor("woT", [128, 2, H], BF16, kind="ExternalInput")
    cos2 = nc.dram_tensor("cos2", [128, S], BF16, kind="ExternalInput")
    sinr = nc.dram_tensor("sinr", [128, S], BF16, kind="ExternalInput")
    onesd = nc.dram_tensor("onesd", [128, MC * GH], BF16, kind="ExternalInput")
    zerosd = nc.dram_tensor("zerosd", [128, S], BF16, kind="ExternalInput")
    outp = nc.dram_tensor("outp", [H, S], F32, kind="ExternalOutput")

    import os as _os
    _repeat = int(_os.environ.get('KERNEL_REPEAT', '1'))
    _mode = _os.environ.get('KERNEL_REPEAT_MODE', 'all')
    with tile.TileContext(nc) as tc:
        with (
            tc.tile_pool(name="const", bufs=1) as const,
            tc.tile_pool(name="persist", bufs=1) as persist,
        ):
            cos_sb = const.tile([128, S], BF16)
            sinr_sb = const.tile([128, S], BF16)
            xT_sb = const.tile([128, KT, S], BF16)
            wqT_sb = const.tile([128, KT, GD], BF16)
            wkT_sb = const.tile([128, KT, GD], BF16)
            wvT_sb = const.tile([128, KT, GD], BF16)
            woT_sb = const.tile([128, 2, H], BF16)

            qT_sb = persist.tile([128, 2, S], BF16)
            # kTz: per-head slots with the other head's 64 rows zeroed so
            # score matmuls run at K=128 (K=64 underruns the PE row rate)
            kTz_sb = persist.tile([128, GH, S], BF16)
            v_sb = persist.tile([128, MC, GH, D + 1], BF16)

            zview = zerosd.rearrange("(a p) s -> p a s", a=2)
            nc.sync.dma_start(kTz_sb[64:128, 0::2, :], zview)
            nc.sync.dma_start(kTz_sb[0:64, 1::2, :], zview)
            for _rep in range(_repeat):
                _do_c = (_rep == 0) or (_mode in ('all', 'c'))
                if _rep == 0 or _mode in ('all', 'b'):
                    # ------------- phase B: projections + rope -------------
                    # DMA order matches first-need order of the PE queue.
                    nc.sync.dma_start(wqT_sb[:], wqT[:])
                    nc.sync.dma_start(wkT_sb[:], wkT[:])
                    _x_prefetched = (_mode == 'all' and _rep > 0)
                    for qb in range(QB):
                        sl = bass.ts(qb, 512)
                        if not _x_prefetched:
                            for kt in range(KT):
                                nc.sync.dma_start(xT_sb[:, kt, sl], xT[:, kt, sl])
                        if qb == 0:
                            nc.sync.dma_start(cos_sb[:], cos2[:])
                            nc.sync.dma_start(sinr_sb[:], sinr[:])
                        elif qb == 1:
                            nc.sync.dma_start(wvT_sb[:], wvT[:])
                        elif qb == 2:
                            nc.sync.dma_start(
                                v_sb[:, :, :, D:D + 1],
                                onesd.rearrange("p (a b o) -> p a b o", a=MC, o=1),
                            )
                            nc.sync.dma_start(woT_sb[:], woT[:])

                    with (
                        tc.tile_pool(name="ptmp", bufs=2) as ptmp,
                        tc.tile_pool(name="ppsum", bufs=6, space="PSUM") as ppsum,
                        tc.tile_pool(name="vpsum", bufs=2, space="PSUM") as vpsum,
                    ):
                        st_tiles = {}

                        def qk_proj_nb(w_sb, which, m, out_sb, nb):
                            # 128 outdims' matmuls for head-pair m over one
                            # 512-col block; Pool stages psum->bf16, DVE
                            # applies rope per block.
                            key = (which, m)
                            if key not in st_tiles:
                                st_tiles[key] = ptmp.tile(
                                    [128, S], BF16, tag="stage",
                                    name=f"st_{_rep}_{which}_{m}")
                            st = st_tiles[key]
                            sl = bass.ts(nb, 512)
                            ps = ppsum.tile([128, 512], F32, tag="pp")
                            for kt in range(KT):
                                nc.tensor.matmul(
                                    ps[:], w_sb[:, kt, bass.ts(m, 128)],
                                    xT_sb[:, kt, sl],
                                    start=(kt == 0), stop=(kt == KT - 1),
                                )
                            nc.scalar.copy(st[:, sl], ps[:])
                            tmpR = ptmp.tile([128, 512], BF16, tag="rot",
                                             name=f"tr_{_rep}_{which}_{m}_{nb}")
                            nc.vector.tensor_mul(tmpR[0:32], st[32:64, sl], sinr_sb[32:64, sl])
                            nc.vector.tensor_mul(tmpR[32:64], st[0:32, sl], sinr_sb[0:32, sl])
                            nc.gpsimd.tensor_mul(tmpR[64:96], st[96:128, sl], sinr_sb[96:128, sl])
                            nc.gpsimd.tensor_mul(tmpR[96:128], st[64:96, sl], sinr_sb[64:96, sl])
                            tmpC = ptmp.tile([128, 512], BF16, tag="cosp",
                                             name=f"tc_{_rep}_{which}_{m}_{nb}")
                            nc.vector.tensor_mul(tmpC[:], st[:, sl], cos_sb[:, sl])
                            if which == "q":
                                nc.vector.tensor_add(out_sb[:, m, sl], tmpC[:], tmpR[:])
                            else:
                                nc.vector.tensor_add(
                                    kTz_sb[0:64, 2 * m, sl], tmpC[0:64], tmpR[0:64])
                                nc.vector.tensor_add(
                                    kTz_sb[64:128, 2 * m + 1, sl], tmpC[64:128], tmpR[64:128])

                        def v_proj_pair(mc):
                            # two seq chunks share one PSUM bank (256 cols
                            # each); natural [seq, dims] layout + ones col
                            ps = vpsum.tile([128, 2, GD], F32, tag="vp")
                            for half in range(2):
                                for kt in range(KT):
                                    nc.tensor.matmul(
                                        ps[:, half, :],
                                        xT_sb[:, kt, bass.ts(mc + half, 128)],
                                        wvT_sb[:, kt, :],
                                        start=(kt == 0), stop=(kt == KT - 1),
                                    )
                            nc.scalar.copy(
                                v_sb[:, mc:mc + 2, :, 0:D],
                                ps.rearrange("p a (h d) -> p a h d", h=GH),
                            )

                        # m=0 q/k and v interleaved per 512-col xT chunk so the
                        # PE has ~5us of work per ~1MB chunk as DMAs land.
                        # For reps > 0 in 'all' mode, V was already computed
                        # at the previous rep's attention tail (phase C flush).
                        _inline_v = (_rep == 0 or _mode != 'all')
                        for nb in range(QB):
                            qk_proj_nb(wqT_sb, "q", 0, qT_sb, nb)
                            qk_proj_nb(wkT_sb, "k", 0, qT_sb, nb)
                            if _inline_v:
                                v_proj_pair(4 * nb)
                                v_proj_pair(4 * nb + 2)
                        for nb in range(QB):
                            qk_proj_nb(wqT_sb, "q", 1, qT_sb, nb)
                            qk_proj_nb(wkT_sb, "k", 1, qT_sb, nb)

                # ------------- phase C: attention + output projection -------------
                # Software-pipelined: iteration i computes scores+exp for item i
                # and the attn@V / normalize for item i-1.
                if not _do_c:
                    continue
                with (
                    tc.tile_pool(name="cpersist", bufs=1) as cpersist,
                    tc.tile_pool(name="es", bufs=2) as es_pool,
                    tc.tile_pool(name="esa3", bufs=3) as esa_pool,
                    tc.tile_pool(name="atmp", bufs=4) as atmp,
                    tc.tile_pool(name="osb", bufs=3) as osb_pool,
                    tc.tile_pool(name="spsum", bufs=2, space="PSUM") as spsum,
                    tc.tile_pool(name="smallps", bufs=2, space="PSUM") as smallps,
                ):
                    aoT_sb = cpersist.tile([128, 2, S], BF16)

                    SG = (3, 3, 2, 3, 3, 2)   # score-group sizes (kc chunks)

                    def score_group(qb, h, i, g, es_tiles):
                        qsl = bass.ts(qb, 512)
                        kc0 = sum(SG[:g])
                        gsz = SG[g]
                        es = es_tiles[0] if kc0 < MC // 2 else es_tiles[1]
                        off = 0 if kc0 < MC // 2 else MC // 2
                        sp = spsum.tile([128, 3, 512], F32, tag="sp",
                                        name=f"sp_{_rep}_{i}_{kc0}")
                        for j in range(gsz):
                            kc = kc0 + j
                            nc.tensor.matmul(
                                sp[:, j, :],
                                kTz_sb[:, h, bass.ts(kc, 128)],
                                qT_sb[:, h // 2, qsl],
                                start=True, stop=True,
                            )
                        nc.scalar.activation(
                            es[:, kc0 - off:kc0 - off + gsz, :],
                            sp[:, 0:gsz, :],
                            EXP, scale=SCALE,
                        )

                    def attnv_chunk(h, es_tiles, ao, kc0):
                        # 4 kc of the attn@V accumulation for the prev item
                        for kc in range(kc0, kc0 + 4):
                            eshalf = es_tiles[0] if kc < MC // 2 else es_tiles[1]
                            nc.tensor.matmul(
                                ao[:], v_sb[:, kc, h, :],
                                eshalf[:, kc % (MC // 2), :],
                                start=(kc == 0), stop=(kc == MC - 1),
                            )

                    def norm(qb, h, ao):
                        qsl = bass.ts(qb, 512)
                        hc, hr = h // 2, (h % 2) * 64
                        rcp = atmp.tile([1, 512], F32, tag="rcp")
                        nc.vector.reciprocal(rcp[:], ao[D:D + 1, :])
                        bsb = atmp.tile([D, 512], F32, tag="bsb")
                        nc.gpsimd.partition_broadcast(bsb[:], rcp[:])
                        nc.vector.tensor_mul(
                            aoT_sb[hr:hr + 64, hc, qsl], ao[0:D, :], bsb[:],
                        )

                    def oproj(qb):
                        # transposed output: partial^T[hid, seq]
                        qsl = bass.ts(qb, 512)
                        for hc8 in range(8):
                            ps = smallps.tile([128, 512], F32, tag="ao",
                                              name=f"op_{_rep}_{qb}_{hc8}")
                            for kt in range(2):
                                nc.tensor.matmul(
                                    ps[:], woT_sb[:, kt, bass.ts(hc8, 128)],
                                    aoT_sb[:, kt, qsl],
                                    start=(kt == 0), stop=(kt == 1),
                                )
                            o_sb = osb_pool.tile([128, 512], F32, tag="ot")
                            nc.vector.tensor_copy(o_sb[:], ps[:])
                            nc.sync.dma_start(
                                outp[bass.ts(hc8, 128), qsl], o_sb[:],
                            )

                    def item_stream(qb, h, i, prev):
                        # Emit item i's score groups with the previous item's
                        # attn@V chunks interleaved so the PE fills its
                        # exp-backpressure waits; prev's oproj (if it ended a
                        # query block) goes at the end of this item's stream.
                        es_tiles = (
                            esa_pool.tile([128, MC // 2, 512], BF16, tag="esa",
                                          name=f"esa_{_rep}_{i}"),
                            es_pool.tile([128, MC // 2, 512], BF16, tag="esb",
                                         name=f"esb_{_rep}_{i}"),
                        )
                        if prev is None:
                            for g in range(6):
                                score_group(qb, h, i, g, es_tiles)
                            return es_tiles
                        pqb, ph, pes, pi = prev
                        ao = smallps.tile([D + 1, 512], F32, tag="ao",
                                          name=f"ao_{_rep}_{pi}")
                        score_group(qb, h, i, 0, es_tiles)
                        score_group(qb, h, i, 1, es_tiles)
                        attnv_chunk(ph, pes, ao, 0)
                        score_group(qb, h, i, 2, es_tiles)
                        attnv_chunk(ph, pes, ao, 4)
                        score_group(qb, h, i, 3, es_tiles)
                        attnv_chunk(ph, pes, ao, 8)
                        score_group(qb, h, i, 4, es_tiles)
                        attnv_chunk(ph, pes, ao, 12)
                        score_group(qb, h, i, 5, es_tiles)
                        norm(pqb, ph, ao)
                        if ph == GH - 1 and pqb > 0:
                            oproj(pqb - 1)  # emitted one qb late: aoT settled
                        return es_tiles

                    items = [(qb, h) for qb in range(QB) for h in range(GH)]
                    prev = None
                    for i, (qb, h) in enumerate(items):
                        es = item_stream(qb, h, i, prev)
                        prev = (qb, h, es, i)
                    # flush: attn@V + norm of the last item, then the last two
                    # query blocks' output projections
                    pqb, ph, pes, pi = prev
                    ao = smallps.tile([D + 1, 512], F32, tag="ao",
                                      name=f"ao_{_rep}_{pi}")
                    for kc0 in (0, 4, 8, 12):
                        attnv_chunk(ph, pes, ao, kc0)
                    norm(pqb, ph, ao)
                    oproj(QB - 2)
                    oproj(QB - 1)
                    if _mode == 'all' and _rep + 1 < _repeat:
                        # prefetch next rep's x and compute its V projection
                        # here: fills the tail's norm/oproj wait bubbles and
                        # shrinks the next rep's serial projection phase
                        for qb2 in range(QB):
                            sl2 = bass.ts(qb2, 512)
                            for kt in range(KT):
                                nc.sync.dma_start(
                                    xT_sb[:, kt, sl2], xT[:, kt, sl2])
                        for mc in range(0, MC, 2):
                            ps = smallps.tile([128, 2, GD], F32, tag="ao",
                                              name=f"vt_{_rep}_{mc}")
                            for half in range(2):
                                for kt in range(KT):
                                    nc.tensor.matmul(
                                        ps[:, half, :],
                                        xT_sb[:, kt, bass.ts(mc + half, 128)],
                                        wvT_sb[:, kt, :],
                                        start=(kt == 0), stop=(kt == KT - 1),
                                    )
                            nc.scalar.copy(
                                v_sb[:, mc:mc + 2, :, 0:D],
                                ps.rearrange("p a (h d) -> p a h d", h=GH),
                            )

    nc.compile()
    return nc


_NC_CACHE = None
_last_in_maps = None


def _get_nc():
    global _NC_CACHE
    if _NC_CACHE is None:
        _NC_CACHE = _build_nc()
    return _NC_CACHE


def make_in_maps(x, Wq, Wk, Wv, Wo):
    cos2, sinr = _rope_tables()
    bf = ml_dtypes.bfloat16

    def fold(a):  # [X, F] with X=128*KTI -> [128, KTI, F]
        kti = a.shape[0] // 128
        return np.ascontiguousarray(
            a.reshape(kti, 128, -1).transpose(1, 0, 2)).astype(bf)

    in_maps = []
    for core in range(8):
        b, g = core // 4, core % 4
        rows = slice(g * GD, (g + 1) * GD)
        in_maps.append({
            "xT": fold(np.ascontiguousarray(x[b].T)),          # (1024, S)
            "wqT": fold(np.ascontiguousarray(Wq[rows].T)),     # (1024, 256)
            "wkT": fold(np.ascontiguousarray(Wk[rows].T)),
            "wvT": fold(np.ascontiguousarray(Wv[rows].T)),
            "woT": fold(np.ascontiguousarray(Wo[:, rows].T)),  # (256, 1024)
            "cos2": cos2,
            "sinr": sinr,
            "onesd": np.ones((128, MC * GH), dtype=bf),
            "zerosd": np.zeros((128, S), dtype=bf),
        })
    return in_maps


def kernel(x, Wq, Wk, Wv, Wo):
    x = np.asarray(x, dtype=np.float32)
    Wq = np.asarray(Wq, dtype=np.float32)
    Wk = np.asarray(Wk, dtype=np.float32)
    Wv = np.asarray(Wv, dtype=np.float32)
    Wo = np.asarray(Wo, dtype=np.float32)

    global _last_in_maps
    in_maps = make_in_maps(x, Wq, Wk, Wv, Wo)
    _last_in_maps = in_maps
    nc = _get_nc()
    # first execution after NEFF load can read junk SBUF on a core (axon
    # first-run glitch); run twice and keep the warm result
    run_bass_kernel_spmd(nc, in_maps, core_ids=list(range(8)))
    res = run_bass_kernel_spmd(nc, in_maps, core_ids=list(range(8)))
    out = np.zeros((B, S, H), dtype=np.float32)
    for core in range(8):
        out[core // 4] += res.results[core]["outp"].T
    return out


# revision 19
# speedup vs baseline: 1.2876x; 1.2876x over previous
"""Multi-head attention (B=2, S=2048, H=1024, 16 heads, RoPE) on 8 trn2 cores.

Sharding: core = (batch b, head-group g); b = core // 4, g = core % 4.
Each core computes 4 heads' attention for one batch and a partial output
projection; the host sums the 4 partials per batch.

All operand data is bf16 (PE runs 1 cycle/row at any K, so no zero-padded
K=128 trick is needed for the 64-dim per-head score contractions); PSUM
accumulation stays fp32 and the output partials are written fp32.
Attention scores are computed directly in transposed [k, q] layout so the
attn @ V contraction needs no transposes; softmax normalization is deferred:
V carries an extra ones-column so the attention matmul also produces the
softmax denominator, and gpsimd partition_broadcast replicates 1/rowsum
across partitions for the final scale.

Schedule: phase B emits q(m=0), k(m=0), v, q(m=1), k(m=1) so phase C's first
score items (heads 0/1) have their rope chains complete by the time the PE
reaches them; xT is DMA'd in per-512-column chunks interleaved with the
weight loads so the first projection matmuls start ~4us in. xT and all
weights live in persistent SBUF tiles so the next rep's loads overlap this
rep's attention phase instead of waiting for pool reuse.
"""
import sys

import numpy as np
import ml_dtypes

sys.path.insert(0, "/opt/trn_rl_repo")

import concourse.bass as bass  # noqa: E402
import concourse.mybir as mybir  # noqa: E402
import concourse.tile as tile  # noqa: E402
from concourse import bacc  # noqa: E402
from concourse.bass_utils import run_bass_kernel_spmd  # noqa: E402

F32 = mybir.dt.float32
BF16 = mybir.dt.bfloat16
EXP = mybir.ActivationFunctionType.Exp

B, S, H = 2, 2048, 1024
NH, D = 16, 64                  # heads, head dim
GH = 4                          # heads per core (group)
GD = GH * D                     # 256 out dims per core
KT = H // 128                   # 8 contraction tiles for projections
MC = S // 128                   # 16 seq chunks of 128
QB = S // 512                   # 4 query blocks of 512
ROPE_BASE = 10000.0
SCALE = D ** -0.5


def _rope_tables():
    inv_freq = 1.0 / (ROPE_BASE ** (np.arange(0, D, 2, dtype=np.float64) / D))
    t = np.arange(S, dtype=np.float64)
    freqs = np.outer(t, inv_freq)                     # (S, 32)
    emb = np.concatenate([freqs, freqs], axis=-1)     # (S, 64)
    cosT = np.cos(emb).T.astype(np.float32)           # (64, S)
    sinT = np.sin(emb).T.astype(np.float32)           # (64, S)
    # sinrs is laid out at SOURCE row positions so that tmp[dest] =
    # st[src] * sinrs[src] has equal input base partitions (ISA rule):
    #   dest 0-31  <- src 32-63: factor -sin[dest]; stored at rows 32-63
    #   dest 32-63 <- src 0-31:  factor +sin[dest]; stored at rows 0-31
    # (sinT rows 0-31 and 32-63 are identical, so signs are what matter)
    sinrs = np.empty_like(sinT)
    sinrs[0:32] = sinT[0:32]
    sinrs[32:64] = -sinT[32:64]
    cos2 = np.tile(cosT, (2, 1))                      # (128, S) two heads/chunk
    sinr2 = np.tile(sinrs, (2, 1))
    bf = ml_dtypes.bfloat16
    return np.ascontiguousarray(cos2).astype(bf), np.ascontiguousarray(sinr2).astype(bf)


def _build_nc():
    nc = bacc.Bacc("TRN2", target_bir_lowering=False)
    xT = nc.dram_tensor("xT", [128, KT, S], BF16, kind="ExternalInput")
    wqT = nc.dram_tensor("wqT", [128, KT, GD], BF16, kind="ExternalInput")
    wkT = nc.dram_tensor("wkT", [128, KT, GD], BF16, kind="ExternalInput")
    wvT = nc.dram_tensor("wvT", [128, KT, GD], BF16, kind="ExternalInput")
    woT = nc.dram_tensor("woT", [128, 2, H], BF16, kind="ExternalInput")
    cos2 = nc.dram_tensor("cos2", [128, S], BF16, kind="ExternalInput")
    sinr = nc.dram_tensor("sinr", [128, S], BF16, kind="ExternalInput")
    onesd = nc.dram_tensor("onesd", [128, MC * GH], BF16, kind="ExternalInput")
    zerosd = nc.dram_tensor("zerosd", [128, S], BF16, kind="ExternalInput")
    outp = nc.dram_tensor("outp", [H, S], F32, kind="ExternalOutput")

    import os as _os
    _repeat = int(_os.environ.get('KERNEL_REPEAT', '1'))
    _mode = _os.environ.get('KERNEL_REPEAT_MODE', 'all')
    with tile.TileContext(nc) as tc:
        with (
            tc.tile_pool(name="const", bufs=1) as const,
            tc.tile_pool(name="persist", bufs=1) as persist,
        ):
            cos_sb = const.tile([128, S], BF16)
            sinr_sb = const.tile([128, S], BF16)
            xT_sb = const.tile([128, KT, S], BF16)
            wqT_sb = const.tile([128, KT, GD], BF16)
            wkT_sb = const.tile([128, KT, GD], BF16)
            wvT_sb = const.tile([128, KT, GD], BF16)
            woT_sb = const.tile([128, 2, H], BF16)

            qT_sb = persist.tile([128, 2, S], BF16)
            # kTz: per-head slots with the other head's 64 rows zeroed so
            # score matmuls run at K=128 (K=64 underruns the PE row rate)
            kTz_sb = persist.tile([128, GH, S], BF16)
            v_sb = persist.tile([128, MC, GH, D + 1], BF16)

            zview = zerosd.rearrange("(a p) s -> p a s", a=2)
            nc.sync.dma_start(kTz_sb[64:128, 0::2, :], zview)
            nc.sync.dma_start(kTz_sb[0:64, 1::2, :], zview)
            for _rep in range(_repeat):
                _do_c = (_rep == 0) or (_mode in ('all', 'c'))
                if _rep == 0 or _mode in ('all', 'b'):
                    # ------------- phase B: projections + rope -------------
                    # DMA order matches first-need order of the PE queue.
                    nc.sync.dma_start(wqT_sb[:], wqT[:])
                    nc.sync.dma_start(wkT_sb[:], wkT[:])
                    _x_prefetched = (_mode == 'all' and _rep > 0)
                    for qb in range(QB):
                        sl = bass.ts(qb, 512)
                        if not _x_prefetched:
                            for kt in range(KT):
                                nc.sync.dma_start(xT_sb[:, kt, sl], xT[:, kt, sl])
                        if qb == 0:
                            nc.sync.dma_start(cos_sb[:], cos2[:])
                            nc.sync.dma_start(sinr_sb[:], sinr[:])
                        elif qb == 1:
                            nc.sync.dma_start(wvT_sb[:], wvT[:])
                        elif qb == 2:
                            nc.sync.dma_start(
                                v_sb[:, :, :, D:D + 1],
                                onesd.rearrange("p (a b o) -> p a b o", a=MC, o=1),
                            )
                            nc.sync.dma_start(woT_sb[:], woT[:])

                    with (
                        tc.tile_pool(name="ptmp", bufs=2) as ptmp,
                        tc.tile_pool(name="ppsum", bufs=6, space="PSUM") as ppsum,
                        tc.tile_pool(name="vpsum", bufs=2, space="PSUM") as vpsum,
                    ):
                        st_tiles = {}

                        def qk_proj_nb(w_sb, which, m, out_sb, nb):
                            # 128 outdims' matmuls for head-pair m over one
                            # 512-col block; Pool stages psum->bf16, DVE
                            # applies rope per block.
                            key = (which, m)
                            if key not in st_tiles:
                                st_tiles[key] = ptmp.tile(
                                    [128, S], BF16, tag="stage",
                                    name=f"st_{_rep}_{which}_{m}")
                            st = st_tiles[key]
                            sl = bass.ts(nb, 512)
                            ps = ppsum.tile([128, 512], F32, tag="pp")
                            for kt in range(KT):
                                nc.tensor.matmul(
                                    ps[:], w_sb[:, kt, bass.ts(m, 128)],
                                    xT_sb[:, kt, sl],
                                    start=(kt == 0), stop=(kt == KT - 1),
                                )
                            nc.scalar.copy(st[:, sl], ps[:])
                            tmpR = ptmp.tile([128, 512], BF16, tag="rot",
                                             name=f"tr_{_rep}_{which}_{m}_{nb}")
                            nc.vector.tensor_mul(tmpR[0:32], st[32:64, sl], sinr_sb[32:64, sl])
                            nc.vector.tensor_mul(tmpR[32:64], st[0:32, sl], sinr_sb[0:32, sl])
                            nc.gpsimd.tensor_mul(tmpR[64:96], st[96:128, sl], sinr_sb[96:128, sl])
                            nc.gpsimd.tensor_mul(tmpR[96:128], st[64:96, sl], sinr_sb[64:96, sl])
                            tmpC = ptmp.tile([128, 512], BF16, tag="cosp",
                                             name=f"tc_{_rep}_{which}_{m}_{nb}")
                            nc.vector.tensor_mul(tmpC[:], st[:, sl], cos_sb[:, sl])
                            if which == "q":
                                nc.vector.tensor_add(out_sb[:, m, sl], tmpC[:], tmpR[:])
                            else:
                                nc.vector.tensor_add(
                                    kTz_sb[0:64, 2 * m, sl], tmpC[0:64], tmpR[0:64])
                                nc.vector.tensor_add(
                                    kTz_sb[64:128, 2 * m + 1, sl], tmpC[64:128], tmpR[64:128])

                        def v_proj_pair(mc):
                            # two seq chunks share one PSUM bank (256 cols
                            # each); natural [seq, dims] layout + ones col
                            ps = vpsum.tile([128, 2, GD], F32, tag="vp")
                            for half in range(2):
                                for kt in range(KT):
                                    nc.tensor.matmul(
                                        ps[:, half, :],
                                        xT_sb[:, kt, bass.ts(mc + half, 128)],
                                        wvT_sb[:, kt, :],
                                        start=(kt == 0), stop=(kt == KT - 1),
                                    )
                            nc.scalar.copy(
                                v_sb[:, mc:mc + 2, :, 0:D],
                                ps.rearrange("p a (h d) -> p a h d", h=GH),
                            )

                        # m=0 q/k and v interleaved per 512-col xT chunk so the
                        # PE has ~5us of work per ~1MB chunk as DMAs land.
                        # For reps > 0 in 'all' mode, V was already computed
                        # at the previous rep's attention tail (phase C flush).
                        _inline_v = (_rep == 0 or _mode != 'all')
                        for nb in range(QB):
                            qk_proj_nb(wqT_sb, "q", 0, qT_sb, nb)
                            qk_proj_nb(wkT_sb, "k", 0, qT_sb, nb)
                            if _inline_v:
                                v_proj_pair(4 * nb)
                                v_proj_pair(4 * nb + 2)
                        for nb in range(QB):
                            qk_proj_nb(wqT_sb, "q", 1, qT_sb, nb)
                            qk_proj_nb(wkT_sb, "k", 1, qT_sb, nb)

                # ------------- phase C: attention + output projection -------------
                # Software-pipelined: iteration i computes scores+exp for item i
                # and the attn@V / normalize for item i-1.
                if not _do_c:
                    continue
                with (
                    tc.tile_pool(name="cpersist", bufs=1) as cpersist,
                    tc.tile_pool(name="es", bufs=3) as es_pool,
                    tc.tile_pool(name="esa3", bufs=4) as esa_pool,
                    tc.tile_pool(name="atmp", bufs=6) as atmp,
                    tc.tile_pool(name="osb", bufs=4) as osb_pool,
                    tc.tile_pool(name="spsum", bufs=2, space="PSUM") as spsum,
                    tc.tile_pool(name="smallps", bufs=2, space="PSUM") as smallps,
                ):
                    aoT_sb = cpersist.tile([128, 2, S], BF16)

                    SG = (3, 3, 2, 3, 3, 2)   # score-group sizes (kc chunks)

                    def score_group(qb, h, i, g, es_tiles):
                        qsl = bass.ts(qb, 512)
                        kc0 = sum(SG[:g])
                        gsz = SG[g]
                        es = es_tiles[0] if kc0 < MC // 2 else es_tiles[1]
                        off = 0 if kc0 < MC // 2 else MC // 2
                        sp = spsum.tile([128, 3, 512], F32, tag="sp",
                                        name=f"sp_{_rep}_{i}_{kc0}")
                        for j in range(gsz):
                            kc = kc0 + j
                            nc.tensor.matmul(
                                sp[:, j, :],
                                kTz_sb[:, h, bass.ts(kc, 128)],
                                qT_sb[:, h // 2, qsl],
                                start=True, stop=True,
                            )
                        nc.scalar.activation(
                            es[:, kc0 - off:kc0 - off + gsz, :],
                            sp[:, 0:gsz, :],
                            EXP, scale=SCALE,
                        )

                    def attnv_chunk(h, es_tiles, ao, kc0):
                        # 4 kc of the attn@V accumulation for the prev item
                        for kc in range(kc0, kc0 + 4):
                            eshalf = es_tiles[0] if kc < MC // 2 else es_tiles[1]
                            nc.tensor.matmul(
                                ao[:], v_sb[:, kc, h, :],
                                eshalf[:, kc % (MC // 2), :],
                                start=(kc == 0), stop=(kc == MC - 1),
                            )

                    def norm(qb, h, ao):
                        qsl = bass.ts(qb, 512)
                        hc, hr = h // 2, (h % 2) * 64
                        rcp = atmp.tile([1, 512], F32, tag="rcp")
                        nc.vector.reciprocal(rcp[:], ao[D:D + 1, :])
                        bsb = atmp.tile([D, 512], F32, tag="bsb")
                        nc.gpsimd.partition_broadcast(bsb[:], rcp[:])
                        nc.vector.tensor_mul(
                            aoT_sb[hr:hr + 64, hc, qsl], ao[0:D, :], bsb[:],
                        )

                    def oproj(qb):
                        # transposed output: partial^T[hid, seq]
                        qsl = bass.ts(qb, 512)
                        for hc8 in range(8):
                            ps = smallps.tile([128, 512], F32, tag="ao",
                                              name=f"op_{_rep}_{qb}_{hc8}")
                            for kt in range(2):
                                nc.tensor.matmul(
                                    ps[:], woT_sb[:, kt, bass.ts(hc8, 128)],
                                    aoT_sb[:, kt, qsl],
                                    start=(kt == 0), stop=(kt == 1),
                                )
                            o_sb = osb_pool.tile([128, 512], F32, tag="ot")
                            nc.vector.tensor_copy(o_sb[:], ps[:])
                            nc.sync.dma_start(
                                outp[bass.ts(hc8, 128), qsl], o_sb[:],
                            )

                    def item_stream(qb, h, i, prev):
                        # Emit item i's score groups with the previous item's
                        # attn@V chunks interleaved so the PE fills its
                        # exp-backpressure waits; prev's oproj (if it ended a
                        # query block) goes at the end of this item's stream.
                        es_tiles = (
                            esa_pool.tile([128, MC // 2, 512], BF16, tag="esa",
                                          name=f"esa_{_rep}_{i}"),
                            es_pool.tile([128, MC // 2, 512], BF16, tag="esb",
                                         name=f"esb_{_rep}_{i}"),
                        )
                        if prev is None:
                            for g in range(6):
                                score_group(qb, h, i, g, es_tiles)
                            return es_tiles
                        pqb, ph, pes, pi = prev
                        ao = smallps.tile([D + 1, 512], F32, tag="ao",
                                          name=f"ao_{_rep}_{pi}")
                        score_group(qb, h, i, 0, es_tiles)
                        score_group(qb, h, i, 1, es_tiles)
                        attnv_chunk(ph, pes, ao, 0)
                        attnv_chunk(ph, pes, ao, 4)
                        score_group(qb, h, i, 2, es_tiles)
                        score_group(qb, h, i, 3, es_tiles)
                        attnv_chunk(ph, pes, ao, 8)
                        attnv_chunk(ph, pes, ao, 12)
                        score_group(qb, h, i, 4, es_tiles)
                        score_group(qb, h, i, 5, es_tiles)
                        norm(pqb, ph, ao)
                        if ph == GH - 1 and pqb > 0:
                            oproj(pqb - 1)  # emitted one qb late: aoT settled
                        return es_tiles

                    items = [(qb, h) for qb in range(QB) for h in range(GH)]
                    prev = None
                    for i, (qb, h) in enumerate(items):
                        es = item_stream(qb, h, i, prev)
                        prev = (qb, h, es, i)
                    # flush: attn@V + norm of the last item, then the last two
                    # query blocks' output projections
                    pqb, ph, pes, pi = prev
                    ao = smallps.tile([D + 1, 512], F32, tag="ao",
                                      name=f"ao_{_rep}_{pi}")
                    for kc0 in (0, 4, 8, 12):
                        attnv_chunk(ph, pes, ao, kc0)
                    norm(pqb, ph, ao)
                    oproj(QB - 2)
                    oproj(QB - 1)
                    if _mode == 'all' and _rep + 1 < _repeat:
                        # prefetch next rep's x and compute its V projection
                        # here: fills the tail's norm/oproj wait bubbles and
                        # shrinks the next rep's serial projection phase
                        for qb2 in range(QB):
                            sl2 = bass.ts(qb2, 512)
                            for kt in range(KT):
                                nc.sync.dma_start(
                                    xT_sb[:, kt, sl2], xT[:, kt, sl2])
                        for mc in range(0, MC, 2):
                            ps = smallps.tile([128, 2, GD], F32, tag="ao",
                                              name=f"vt_{_rep}_{mc}")
                            for half in range(2):
                                for kt in range(KT):
                                    nc.tensor.matmul(
                                        ps[:, half, :],
                                        xT_sb[:, kt, bass.ts(mc + half, 128)],
                                        wvT_sb[:, kt, :],
                                        start=(kt == 0), stop=(kt == KT - 1),
                                    )
                            nc.scalar.copy(
                                v_sb[:, mc:mc + 2, :, 0:D],
                                ps.rearrange("p a (h d) -> p a h d", h=GH),
                            )

    nc.compile()
    return nc


_NC_CACHE = None
_last_in_maps = None


def _get_nc():
    global _NC_CACHE
    if _NC_CACHE is None:
        _NC_CACHE = _build_nc()
    return _NC_CACHE


def make_in_maps(x, Wq, Wk, Wv, Wo):
    cos2, sinr = _rope_tables()
    bf = ml_dtypes.bfloat16

    def fold(a):  # [X, F] with X=128*KTI -> [128, KTI, F]
        kti = a.shape[0] // 128
        return np.ascontiguousarray(
            a.reshape(kti, 128, -1).transpose(1, 0, 2)).astype(bf)

    in_maps = []
    for core in range(8):
        b, g = core // 4, core % 4
        rows = slice(g * GD, (g + 1) * GD)
        in_maps.append({
            "xT": fold(np.ascontiguousarray(x[b].T)),          # (1024, S)
            "wqT": fold(np.ascontiguousarray(Wq[rows].T)),     # (1024, 256)
            "wkT": fold(np.ascontiguousarray(Wk[rows].T)),
            "wvT": fold(np.ascontiguousarray(Wv[rows].T)),
            "woT": fold(np.ascontiguousarray(Wo[:, rows].T)),  # (256, 1024)
            "cos2": cos2,
            "sinr": sinr,
            "onesd": np.ones((128, MC * GH), dtype=bf),
            "zerosd": np.zeros((128, S), dtype=bf),
        })
    return in_maps


def kernel(x, Wq, Wk, Wv, Wo):
    x = np.asarray(x, dtype=np.float32)
    Wq = np.asarray(Wq, dtype=np.float32)
    Wk = np.asarray(Wk, dtype=np.float32)
    Wv = np.asarray(Wv, dtype=np.float32)
    Wo = np.asarray(Wo, dtype=np.float32)

    global _last_in_maps
    in_maps = make_in_maps(x, Wq, Wk, Wv, Wo)
    _last_in_maps = in_maps
    nc = _get_nc()
    # first execution after NEFF load can read junk SBUF on a core (axon
    # first-run glitch); run twice and keep the warm result
    run_bass_kernel_spmd(nc, in_maps, core_ids=list(range(8)))
    res = run_bass_kernel_spmd(nc, in_maps, core_ids=list(range(8)))
    out = np.zeros((B, S, H), dtype=np.float32)
    for core in range(8):
        out[core // 4] += res.results[core]["outp"].T
    return out
